# revision 19
# baseline (speedup 1.0000x reference)
"""Multi-head attention (B=2, L=4096, C=512, H=8, Dh=64) on 8 trn2 cores.

Sharding: data-parallel over batch (4 cores per batch element) x
tensor-parallel over heads (2 heads per core). Each core computes a partial
output projection; the host sums the 4 partials per batch element and adds
the bias.

Per-core kernel (scores never hit HBM):
  - inputs: xT [512, 4096] (= x[b].T), wq/wk/wv [512, 128] col slices
    (1/sqrt(Dh) folded into wq), wo [128, 512] row slice
  - Q^T, K^T [128, 4096] bf16 (2 heads x 64 rows)
  - V stored per (k-tile, head) as [128 tokens, 64 dh | 1 ones] so the AV
    matmul emits the softmax denominator in output partition 64
  - per q-chunk of 512: both heads' score tiles live in ONE [128, 2, 512]
    PSUM tile (2 banks); the two score matmuls use PE row groups 0:64 and
    64:128 so they run concurrently; ONE ScalarE exp [128, 1024] drains
    PSUM directly to bf16 SBUF (no DVE copy at all)
  - AV: per (kt, head) one K=128 matmul accumulating att [65, 2, 512] PSUM
  - normalize: reciprocal_approx_fast on the denominator row + gpsimd
    partition_broadcast + DVE multiply -> attn bf16
  - out-proj: out[q,:] = attn.T @ wo per 128-row q-tile
"""

import ml_dtypes
import numpy as np

B, L, C, H = 2, 4096, 512, 8
DH = C // H  # 64
P = 128
NCORES = 8
HEADS_PER_CORE = 2
CORES_PER_BATCH = 4

QCHUNK = 512  # q columns per attention block (1 PSUM bank per head)
NQC = L // QCHUNK  # 8
NKT = L // P  # 32 k-tiles
NCC = C // P  # 4 contraction chunks for projections
VW = DH + 1  # 65: V columns + ones column (denominator)

_cached = {}


def _build(reps=1):
    import concourse.mybir as mybir
    import concourse.tile as tile
    from concourse import bacc

    F32 = mybir.dt.float32
    BF16 = mybir.dt.bfloat16
    U16 = mybir.dt.uint16
    EXP = mybir.ActivationFunctionType.Exp
    MULT = mybir.AluOpType.mult
    ADD = mybir.AluOpType.add
    # Schraudolph bf16 exp: bitcast(u16(round(s*A + B))) ~= exp(s)
    SCH_A = 128.0 / float(np.log(2.0))
    SCH_B = 127.0 * 128.0 - 5.59

    nc = bacc.Bacc("TRN2", target_bir_lowering=False, debug=False,
                   num_devices=NCORES)
    xT = nc.dram_tensor("xT", [C, L], BF16, kind="ExternalInput").ap()
    wq = nc.dram_tensor("wq", [C, P], BF16, kind="ExternalInput").ap()
    wk = nc.dram_tensor("wk", [C, P], BF16, kind="ExternalInput").ap()
    wv = nc.dram_tensor("wv", [C, P], BF16, kind="ExternalInput").ap()
    wo = nc.dram_tensor("wo", [P, C], BF16, kind="ExternalInput").ap()
    out = nc.dram_tensor("out", [L, C], BF16, kind="ExternalOutput").ap()

    with tile.TileContext(nc) as tc:
        import contextlib
        loop_cm = tc.For_i(0, reps, 1) if reps > 1 else contextlib.nullcontext()
        with (
            tc.tile_pool(name="persist", bufs=1) as persist,
            tc.tile_pool(name="xpool", bufs=1) as xpool,
            tc.tile_pool(name="ptp", bufs=6) as ptp,
            tc.tile_pool(name="small", bufs=2) as small,
            tc.tile_pool(name="outp", bufs=3) as outp,
            loop_cm,
        ):
            # ---- load inputs ----
            wq_t = persist.tile([P, NCC, P], BF16)
            wk_t = persist.tile([P, NCC, P], BF16)
            wv_t = persist.tile([P, NCC, P], BF16)
            wo_t = persist.tile([P, C], BF16)
            nc.sync.dma_start(wq_t, wq.rearrange("(k p) m -> p k m", p=P))
            nc.sync.dma_start(wk_t, wk.rearrange("(k p) m -> p k m", p=P))
            nc.sync.dma_start(wv_t, wv.rearrange("(k p) m -> p k m", p=P))
            nc.sync.dma_start(wo_t, wo)

            xt = xpool.tile([P, NCC, L], BF16)
            xTr = xT.rearrange("(k p) n -> p k n", p=P)
            for j in range(32):  # many parallel DMAs: first chunk lands fast
                sl = slice(j * (L // 32), (j + 1) * (L // 32))
                nc.sync.dma_start(xt[:, :, sl], xTr[:, :, sl])

            qT = persist.tile([P, L], BF16)
            kT = persist.tile([P, L], BF16)
            # per (k-tile, head): [V_h (64) | ones (1)]
            v_store = persist.tile([P, NKT, HEADS_PER_CORE, VW], BF16)
            attn = persist.tile([P, L], BF16)

            ones_t = small.tile([P, NKT], F32, tag="ones")
            nc.vector.memset(ones_t, 1.0)
            for h in range(HEADS_PER_CORE):
                nc.vector.tensor_copy(v_store[:, :, h, DH], ones_t)

            # ---- attention; Q/K/V projections interleaved into the ----
            # ---- loops so ScalarE exp starts almost immediately     ----
            with (
                tc.tile_pool(name="s_ps", bufs=2, space="PSUM") as s_ps,
                tc.tile_pool(name="a_ps", bufs=1, space="PSUM") as a_ps,
                tc.tile_pool(name="v_ps", bufs=2, space="PSUM") as v_ps,
            ):
                def qk_proj(dst, w_t, j):
                    # one 512-token chunk of Q^T or K^T
                    ps = v_ps.tile([P, C], F32, tag="vo_ps", name="qkps")
                    ps = ps[:, 0:512]
                    for c in range(NCC):
                        nc.tensor.matmul(
                            ps, w_t[:, c, :],
                            xt[:, c, j * 512:(j + 1) * 512],
                            start=(c == 0), stop=(c == NCC - 1),
                        )
                    nc.vector.tensor_copy(dst[:, j * 512:(j + 1) * 512], ps)

                def v_proj(r):
                    # V tile r: [128 tokens, 128 (2 heads x 64)]
                    ps = v_ps.tile([P, C], F32, tag="vo_ps", name="vps")
                    ps = ps[:, 0:P]
                    for c in range(NCC):
                        nc.tensor.matmul(
                            ps, xt[:, c, r * P:(r + 1) * P], wv_t[:, c, :],
                            start=(c == 0), stop=(c == NCC - 1),
                        )
                    for h in range(HEADS_PER_CORE):
                        nc.vector.tensor_copy(
                            v_store[:, r, h, 0:DH],
                            ps[:, h * DH:(h + 1) * DH])

                def out_proj(qc, qt):
                    q0 = qc * QCHUNK + qt * P
                    ps = v_ps.tile([P, C], F32, tag="vo_ps")
                    nc.tensor.matmul(ps, attn[:, q0:q0 + P], wo_t,
                                     start=True, stop=True)
                    osb = outp.tile([P, C], BF16, tag="osb")
                    nc.vector.tensor_copy(osb, ps)
                    nc.sync.dma_start(out[q0:q0 + P, :], osb)

                qk_proj(qT, wq_t, 0)
                qk_proj(kT, wk_t, 0)

                for qc in range(NQC):
                    qsl = slice(qc * QCHUNK, (qc + 1) * QCHUNK)
                    # att rows 0:64 = sum_k P*V_h, row 64 = denominator
                    att = a_ps.tile([P, HEADS_PER_CORE, QCHUNK], F32,
                                    tag="att")
                    pending = []

                    def av(kt, pt_aps):
                        for h in range(HEADS_PER_CORE):
                            nc.tensor.matmul(
                                att[0:VW, h, :],
                                v_store[:, kt, h, :],
                                pt_aps[h],
                                start=(kt == 0), stop=(kt == NKT - 1),
                            )

                    for kt in range(NKT):
                        if qc == 0:
                            if kt % 4 == 0 and kt > 0:
                                qk_proj(kT, wk_t, kt // 4)  # K^T chunk
                            v_proj(kt)  # V tile kt, just before its AV
                        elif kt in (6, 8, 10, 12):
                            # deferred out-projection (late enough that
                            # the previous q-chunk's normalize is done)
                            out_proj(qc - 1, (kt - 6) // 2)
                        if kt == 16 and qc + 1 < NQC:
                            qk_proj(qT, wq_t, qc + 1)  # next chunk's Q^T
                        sps = s_ps.tile([P, HEADS_PER_CORE, QCHUNK], F32,
                                        tag="sps")
                        # scores: head0 on PE rows 0:64, head1 on 64:128
                        # (concurrent row groups); different PSUM banks
                        for h in range(HEADS_PER_CORE):
                            hsl = slice(h * DH, (h + 1) * DH)
                            nc.tensor.matmul(
                                sps[:, h, :],
                                kT[hsl, kt * P:(kt + 1) * P],
                                qT[hsl, qsl],
                                start=True, stop=True,
                            )
                        # exp on ScalarE, PSUM -> SBUF bf16, both heads
                        pt = ptp.tile([P, HEADS_PER_CORE, QCHUNK],
                                      BF16, tag="pt")
                        nc.scalar.activation(pt, sps, EXP)
                        pt_aps = (pt[:, 0, :], pt[:, 1, :])
                        # AV trails by one k-tile so the PE FIFO never
                        # head-blocks on the next q-chunk's att slot
                        pending.append((kt, pt_aps))
                        if len(pending) > 1:
                            av(*pending.pop(0))
                    av(*pending.pop(0))
                    # stage att to SBUF with one copy so the PSUM slot
                    # frees immediately; normalize from SBUF off-path
                    attsb = small.tile([VW, HEADS_PER_CORE, QCHUNK], F32,
                                       tag="attsb")
                    nc.vector.tensor_copy(attsb, att[0:VW])
                    # both heads' denominator rows -> partition 0, one
                    # approx-reciprocal (reciprocal_approx_fast requires
                    # base_partition 0)
                    den2 = small.tile([1, HEADS_PER_CORE, QCHUNK], F32,
                                      tag="den2")
                    nc.vector.tensor_copy(den2, attsb[DH:DH + 1, :, :])
                    recip2 = small.tile([1, HEADS_PER_CORE, QCHUNK], F32,
                                        tag="recip2")
                    nc.vector.reciprocal_approx_fast(recip2, den2)
                    for h in range(HEADS_PER_CORE):
                        hsl = slice(h * DH, (h + 1) * DH)
                        rb = small.tile([DH, QCHUNK], F32, tag="rb")
                        nc.gpsimd.partition_broadcast(rb, recip2[:, h, :])
                        nc.gpsimd.tensor_tensor(
                            attn[hsl, qsl], attsb[0:DH, h, :], rb, MULT)
                for qt in range(QCHUNK // P):
                    out_proj(NQC - 1, qt)

    nc.compile()
    return nc


def _get_nc(reps=1):
    key = f"nc{reps}"
    if key not in _cached:
        _cached[key] = _build(reps)
    return _cached[key]


def _build_in_maps(inputs):
    x = np.asarray(inputs["x"], dtype=np.float32)
    Wq = np.asarray(inputs["Wq"], dtype=np.float32)
    Wk = np.asarray(inputs["Wk"], dtype=np.float32)
    Wv = np.asarray(inputs["Wv"], dtype=np.float32)
    Wo = np.asarray(inputs["Wo"], dtype=np.float32)

    scale = np.float32(1.0 / np.sqrt(DH))
    in_maps = []
    for core in range(NCORES):
        b = core // CORES_PER_BATCH
        j = core % CORES_PER_BATCH
        csl = slice(j * P, (j + 1) * P)
        bf = ml_dtypes.bfloat16
        in_maps.append({
            "xT": np.ascontiguousarray(x[b].T.astype(bf)),
            "wq": np.ascontiguousarray((Wq[:, csl] * scale).astype(bf)),
            "wk": np.ascontiguousarray(Wk[:, csl].astype(bf)),
            "wv": np.ascontiguousarray(Wv[:, csl].astype(bf)),
            "wo": np.ascontiguousarray(Wo[csl, :].astype(bf)),
        })
    return in_maps


def kernel(x, Wq, Wk, Wv, Wo, bo):
    from concourse import bass_utils

    bo = np.asarray(bo, dtype=np.float32)
    in_maps = _build_in_maps(
        {"x": x, "Wq": Wq, "Wk": Wk, "Wv": Wv, "Wo": Wo})

    res = bass_utils.run_bass_kernel_spmd(
        _get_nc(), in_maps, core_ids=list(range(NCORES)))

    out = np.zeros((B, L, C), dtype=np.float32)
    for core in range(NCORES):
        out[core // CORES_PER_BATCH] += res.results[core]["out"].astype(np.float32)
    out += bo[None, None, :]
    return out


# revision 20
# speedup vs baseline: 1.3681x; 1.3681x over previous
"""Multi-head attention (B=2, L=4096, C=512, H=8, Dh=64) on 8 trn2 cores.

Sharding: data-parallel over batch (4 cores per batch element) x
tensor-parallel over heads (2 heads per core). Each core computes a partial
output projection; the host sums the 4 partials per batch element and adds
the bias.

Per-core kernel (scores never hit HBM):
  - inputs: xT [512, 4096] (= x[b].T), wq/wk/wv [512, 128] col slices
    (1/sqrt(Dh) folded into wq), wo [128, 512] row slice
  - Q^T, K^T [128, 4096] bf16 (2 heads x 64 rows)
  - V stored per (k-tile, head) as [128 tokens, 64 dh | 1 ones] so the AV
    matmul emits the softmax denominator in output partition 64
  - per q-chunk of 512: both heads' score tiles live in ONE [128, 2, 512]
    PSUM tile (2 banks); the two score matmuls use PE row groups 0:64 and
    64:128 so they run concurrently; ONE ScalarE exp [128, 1024] drains
    PSUM directly to bf16 SBUF (no DVE copy at all)
  - AV: per (kt, head) one K=128 matmul accumulating att [65, 2, 512] PSUM
  - normalize: reciprocal_approx_fast on the denominator row + gpsimd
    partition_broadcast + DVE multiply -> attn bf16
  - out-proj: out[q,:] = attn.T @ wo per 128-row q-tile
"""

import ml_dtypes
import numpy as np

B, L, C, H = 2, 4096, 512, 8
DH = C // H  # 64
P = 128
NCORES = 8
HEADS_PER_CORE = 2
CORES_PER_BATCH = 4

QCHUNK = 512  # q columns per attention block (1 PSUM bank per head)
NQC = L // QCHUNK  # 8
NKT = L // P  # 32 k-tiles
NCC = C // P  # 4 contraction chunks for projections
VW = DH + 1  # 65: V columns + ones column (denominator)

_cached = {}


def _build(reps=1):
    import concourse.mybir as mybir
    import concourse.tile as tile
    from concourse import bacc

    F32 = mybir.dt.float32
    BF16 = mybir.dt.bfloat16
    U16 = mybir.dt.uint16
    EXP = mybir.ActivationFunctionType.Exp
    MULT = mybir.AluOpType.mult
    ADD = mybir.AluOpType.add
    # Schraudolph bf16 exp: bitcast(u16(round(s*A + B))) ~= exp(s)
    SCH_A = 128.0 / float(np.log(2.0))
    SCH_B = 127.0 * 128.0 - 5.59

    nc = bacc.Bacc("TRN2", target_bir_lowering=False, debug=False,
                   num_devices=NCORES)
    xT = nc.dram_tensor("xT", [C, L], BF16, kind="ExternalInput").ap()
    wq = nc.dram_tensor("wq", [C, P], BF16, kind="ExternalInput").ap()
    wk = nc.dram_tensor("wk", [C, P], BF16, kind="ExternalInput").ap()
    wv = nc.dram_tensor("wv", [C, P], BF16, kind="ExternalInput").ap()
    wo = nc.dram_tensor("wo", [P, C], BF16, kind="ExternalInput").ap()
    out = nc.dram_tensor("out", [L, C], BF16, kind="ExternalOutput").ap()

    with tile.TileContext(nc) as tc:
        import contextlib
        loop_cm = tc.For_i(0, reps, 1) if reps > 1 else contextlib.nullcontext()
        with (
            tc.tile_pool(name="persist", bufs=1) as persist,
            tc.tile_pool(name="xpool", bufs=1) as xpool,
            tc.tile_pool(name="ptp", bufs=6) as ptp,
            tc.tile_pool(name="small", bufs=2) as small,
            tc.tile_pool(name="outp", bufs=3) as outp,
            loop_cm,
        ):
            # ---- load inputs ----
            wq_t = persist.tile([P, NCC, P], BF16)
            wk_t = persist.tile([P, NCC, P], BF16)
            wv_t = persist.tile([P, NCC, P], BF16)
            wo_t = persist.tile([P, C], BF16)
            nc.sync.dma_start(wq_t, wq.rearrange("(k p) m -> p k m", p=P))
            nc.sync.dma_start(wk_t, wk.rearrange("(k p) m -> p k m", p=P))
            nc.sync.dma_start(wv_t, wv.rearrange("(k p) m -> p k m", p=P))
            nc.sync.dma_start(wo_t, wo)

            xt = xpool.tile([P, NCC, L], BF16)
            xTr = xT.rearrange("(k p) n -> p k n", p=P)
            for j in range(32):  # many parallel DMAs: first chunk lands fast
                sl = slice(j * (L // 32), (j + 1) * (L // 32))
                nc.sync.dma_start(xt[:, :, sl], xTr[:, :, sl])

            qT = persist.tile([P, L], BF16)
            kT = persist.tile([P, L], BF16)
            # per (k-tile, head): [V_h (64) | ones (1)]
            v_store = persist.tile([P, NKT, HEADS_PER_CORE, VW], BF16)
            attn = persist.tile([P, L], BF16)

            ones_t = small.tile([P, NKT], F32, tag="ones")
            nc.vector.memset(ones_t, 1.0)
            for h in range(HEADS_PER_CORE):
                nc.vector.tensor_copy(v_store[:, :, h, DH], ones_t)

            # ---- attention; Q/K/V projections interleaved into the ----
            # ---- loops so ScalarE exp starts almost immediately     ----
            with (
                tc.tile_pool(name="s_ps", bufs=2, space="PSUM") as s_ps,
                tc.tile_pool(name="a_ps", bufs=1, space="PSUM") as a_ps,
                tc.tile_pool(name="v_ps", bufs=2, space="PSUM") as v_ps,
            ):
                def qk_proj(dst, w_t, j):
                    # one 512-token chunk of Q^T or K^T
                    ps = v_ps.tile([P, C], F32, tag="vo_ps", name="qkps")
                    ps = ps[:, 0:512]
                    for c in range(NCC):
                        nc.tensor.matmul(
                            ps, w_t[:, c, :],
                            xt[:, c, j * 512:(j + 1) * 512],
                            start=(c == 0), stop=(c == NCC - 1),
                        )
                    nc.vector.tensor_copy(dst[:, j * 512:(j + 1) * 512], ps)

                def v_proj(r):
                    # V tile r: [128 tokens, 128 (2 heads x 64)]
                    ps = v_ps.tile([P, C], F32, tag="vo_ps", name="vps")
                    ps = ps[:, 0:P]
                    for c in range(NCC):
                        nc.tensor.matmul(
                            ps, xt[:, c, r * P:(r + 1) * P], wv_t[:, c, :],
                            start=(c == 0), stop=(c == NCC - 1),
                        )
                    for h in range(HEADS_PER_CORE):
                        nc.vector.tensor_copy(
                            v_store[:, r, h, 0:DH],
                            ps[:, h * DH:(h + 1) * DH])

                def out_proj(qc, qt):
                    q0 = qc * QCHUNK + qt * P
                    ps = v_ps.tile([P, C], F32, tag="vo_ps")
                    nc.tensor.matmul(ps, attn[:, q0:q0 + P], wo_t,
                                     start=True, stop=True)
                    osb = outp.tile([P, C], BF16, tag="osb")
                    nc.vector.tensor_copy(osb, ps)
                    nc.sync.dma_start(out[q0:q0 + P, :], osb)

                qk_proj(qT, wq_t, 0)
                qk_proj(kT, wk_t, 0)

                for qc in range(NQC):
                    qsl = slice(qc * QCHUNK, (qc + 1) * QCHUNK)
                    # att rows 0:64 = sum_k P*V_h, row 64 = denominator
                    att = a_ps.tile([P, HEADS_PER_CORE, QCHUNK], F32,
                                    tag="att")
                    pending = []

                    def av(kt, pt_aps):
                        for h in range(HEADS_PER_CORE):
                            nc.tensor.matmul(
                                att[0:VW, h, :],
                                v_store[:, kt, h, :],
                                pt_aps[h],
                                start=(kt == 0), stop=(kt == NKT - 1),
                            )

                    for kt in range(NKT):
                        if qc == 0:
                            if kt % 4 == 0 and kt > 0:
                                qk_proj(kT, wk_t, kt // 4)  # K^T chunk
                            v_proj(kt)  # V tile kt, just before its AV
                        elif kt in (6, 8, 10, 12):
                            # deferred out-projection (late enough that
                            # the previous q-chunk's normalize is done)
                            out_proj(qc - 1, (kt - 6) // 2)
                        if kt == 16 and qc + 1 < NQC:
                            qk_proj(qT, wq_t, qc + 1)  # next chunk's Q^T
                        sps = s_ps.tile([P, HEADS_PER_CORE, QCHUNK], F32,
                                        tag="sps")
                        # scores: head0 on PE rows 0:64, head1 on 64:128
                        # (concurrent row groups); different PSUM banks
                        for h in range(HEADS_PER_CORE):
                            hsl = slice(h * DH, (h + 1) * DH)
                            nc.tensor.matmul(
                                sps[:, h, :],
                                kT[hsl, kt * P:(kt + 1) * P],
                                qT[hsl, qsl],
                                start=True, stop=True,
                            )
                        # exp on ScalarE, PSUM -> SBUF bf16, both heads
                        pt = ptp.tile([P, HEADS_PER_CORE, QCHUNK],
                                      BF16, tag="pt")
                        nc.scalar.activation(pt, sps, EXP)
                        pt_aps = (pt[:, 0, :], pt[:, 1, :])
                        # AV trails by one k-tile so the PE FIFO never
                        # head-blocks on the next q-chunk's att slot
                        pending.append((kt, pt_aps))
                        if len(pending) > 1:
                            av(*pending.pop(0))
                    av(*pending.pop(0))
                    # stage att to SBUF with one copy so the PSUM slot
                    # frees immediately; normalize from SBUF off-path
                    attsb = small.tile([VW, HEADS_PER_CORE, QCHUNK], F32,
                                       tag="attsb")
                    nc.vector.tensor_copy(attsb, att[0:VW])
                    # both heads' denominator rows -> partition 0, one
                    # approx-reciprocal (reciprocal_approx_fast requires
                    # base_partition 0)
                    den2 = small.tile([1, HEADS_PER_CORE, QCHUNK], F32,
                                      tag="den2")
                    nc.vector.tensor_copy(den2, attsb[DH:DH + 1, :, :])
                    recip2 = small.tile([1, HEADS_PER_CORE, QCHUNK], F32,
                                        tag="recip2")
                    nc.vector.reciprocal_approx_fast(recip2, den2)
                    for h in range(HEADS_PER_CORE):
                        hsl = slice(h * DH, (h + 1) * DH)
                        rb = small.tile([DH, QCHUNK], F32, tag="rb")
                        nc.gpsimd.partition_broadcast(rb, recip2[:, h, :])
                        nc.vector.tensor_tensor(
                            attn[hsl, qsl], attsb[0:DH, h, :], rb, MULT)
                for qt in range(QCHUNK // P):
                    out_proj(NQC - 1, qt)

    nc.compile()
    return nc


def _get_nc(reps=1):
    key = f"nc{reps}"
    if key not in _cached:
        _cached[key] = _build(reps)
    return _cached[key]


def _build_in_maps(inputs):
    x = np.asarray(inputs["x"], dtype=np.float32)
    Wq = np.asarray(inputs["Wq"], dtype=np.float32)
    Wk = np.asarray(inputs["Wk"], dtype=np.float32)
    Wv = np.asarray(inputs["Wv"], dtype=np.float32)
    Wo = np.asarray(inputs["Wo"], dtype=np.float32)

    scale = np.float32(1.0 / np.sqrt(DH))
    in_maps = []
    for core in range(NCORES):
        b = core // CORES_PER_BATCH
        j = core % CORES_PER_BATCH
        csl = slice(j * P, (j + 1) * P)
        bf = ml_dtypes.bfloat16
        in_maps.append({
            "xT": np.ascontiguousarray(x[b].T.astype(bf)),
            "wq": np.ascontiguousarray((Wq[:, csl] * scale).astype(bf)),
            "wk": np.ascontiguousarray(Wk[:, csl].astype(bf)),
            "wv": np.ascontiguousarray(Wv[:, csl].astype(bf)),
            "wo": np.ascontiguousarray(Wo[csl, :].astype(bf)),
        })
    return in_maps


def kernel(x, Wq, Wk, Wv, Wo, bo):
    from concourse import bass_utils

    bo = np.asarray(bo, dtype=np.float32)
    in_maps = _build_in_maps(
        {"x": x, "Wq": Wq, "Wk": Wk, "Wv": Wv, "Wo": Wo})

    res = bass_utils.run_bass_kernel_spmd(
        _get_nc(), in_maps, core_ids=list(range(NCORES)))

    out = np.zeros((B, L, C), dtype=np.float32)
    for core in range(NCORES):
        out[core // CORES_PER_BATCH] += res.results[core]["out"].astype(np.float32)
    out += bo[None, None, :]
    return out
